# revision 25
# baseline (speedup 1.0000x reference)
"""DoRA linear layer on 8 TRN2 NeuronCores.

out = (magnitude / ||W + s*B@A||_row) * (x @ (W + s*B@A)^T),  s = alpha/rank = 2.

Identity used: the reference's
    dora_out + base_out = mag_norm_scale * (base_out + s * lora_out)
                        = scale_o * (x @ W_adapted^T)
so the kernel runs ONE big fp32r matmul x @ W_ad^T (with the rank-16 term
added as an extra PSUM-accumulated matmul) and a per-out-column scale.

Sharding: data-parallel on tokens (8192 tokens -> 1024/core); W/A/B/mag
replicated. Host side only reshapes/transposes (layout prep) and rounds
fp32 -> fp32r bit format (the dtype the tensor engine consumes).

Row norms of W_ad are computed on-device from the expansion
  ||W + B2@A||^2_row = rowsum(W*W) + 2*rowsum((W@A^T) * B2) + rowsum((B2@G) * B2)
with B2 = s*B, G = A@A^T.  rowsum(W*W) and W@A^T come from one fused fp16
matmul per W^T tile (gram diag + cross term), everything else is tiny.
"""

import sys

sys.path.insert(0, "/opt/trn_rl_repo")

import numpy as np

import concourse.bass as bass  # noqa: F401  (import keeps bass registered)
from concourse import bacc
import concourse.mybir as mybir
from concourse.tile import TileContext
from concourse.bass_utils import run_bass_kernel_spmd
from concourse.masks import make_identity

FP32 = mybir.dt.float32
F32R = mybir.dt.float32r
FP16 = mybir.dt.float16

NCORES = 8
TOK = 8192          # 4 * 2048 tokens
TPC = TOK // NCORES  # 1024 tokens per core
DIN = 4096
DOUT = 4096
RANK = 16
SCALING = 32.0 / 16

NI = DIN // 128     # 32 contraction blocks
NCOL = 8            # output columns of 512
OC = DOUT // NCOL   # 512
NT = TPC // 128     # 8 token tiles per core
H = 8               # ib-chunk size (W-tile working set)
NH = NI // H        # 4 chunks per column


def _round_f32r(x: np.ndarray) -> np.ndarray:
    """Round-to-nearest-even fp32 -> fp32r bit format (11 explicit mantissa
    bits, low 12 bits zero) — matches the PE's own input rounding."""
    u = np.ascontiguousarray(x, dtype=np.float32).view(np.uint32)
    odd = (u >> np.uint32(12)) & np.uint32(1)
    r = (u + np.uint32(0x7FF) + odd) & np.uint32(0xFFFFF000)
    return r.view(np.float32)


def _build_program(ncol_limit=NCOL, skip_prologue=False):
    nc = bacc.Bacc("TRN2", target_bir_lowering=False, debug=False,
                   num_devices=NCORES)

    xt_d = nc.dram_tensor("xt", [128, NI, TPC], FP32, kind="ExternalInput")
    wt_d = nc.dram_tensor("wt", [NCOL, NI, 128, OC], FP32, kind="ExternalInput")
    at_d = nc.dram_tensor("at", [128, NI, RANK], FP32, kind="ExternalInput")
    b2_d = nc.dram_tensor("b2", [128, 32, RANK], FP32, kind="ExternalInput")
    b2t_d = nc.dram_tensor("b2t", [RANK, DOUT], FP32, kind="ExternalInput")
    mag_d = nc.dram_tensor("mag", [128, 32], FP32, kind="ExternalInput")
    out_d = nc.dram_tensor("out", [TPC, DOUT], FP32, kind="ExternalOutput")
    srow_d = nc.dram_tensor("srow_scratch", [NCOL, OC], FP32)

    with TileContext(nc) as tc:
        with (
            tc.tile_pool(name="const", bufs=1) as const,
            tc.tile_pool(name="xtp", bufs=1) as xtp,
            tc.tile_pool(name="wp", bufs=10) as wp,
            tc.tile_pool(name="outp", bufs=10) as outp,
            tc.tile_pool(name="w16p", bufs=6) as w16p,
            tc.tile_pool(name="b2tp", bufs=2) as b2tp,
            tc.tile_pool(name="sbcp", bufs=2) as sbcp,
            tc.tile_pool(name="mp", bufs=3, space="PSUM") as mp,
            tc.tile_pool(name="np", bufs=2, space="PSUM") as npp,
        ):
            ident = const.tile([128, 128], FP32)
            make_identity(nc, ident)

            aT = const.tile([128, NI, RANK], F32R)
            nc.sync.dma_start(aT[:], at_d[:].bitcast(F32R))
            a16 = const.tile([128, NI, RANK], FP16)
            nc.vector.tensor_copy(a16[:], aT[:].bitcast(FP32))
            b2S = const.tile([128, 32, RANK], FP32)
            nc.sync.dma_start(b2S[:], b2_d[:])
            magS = const.tile([128, 32], FP32)
            nc.sync.dma_start(magS[:], mag_d[:])

            # resident x^T  [i_part, i_blk, tok] — four tiles so consumers
            # of early i-blocks need not wait for the whole 16 MiB load
            xTq = []
            for q in range(4):
                xq = xtp.tile([128, 8, TPC], F32R, name=f"xTq{q}")
                nc.sync.dma_start(xq[:], xt_d[:, q * 8:(q + 1) * 8, :].bitcast(F32R))
                xTq.append(xq)

            def xT(ib):
                return xTq[ib // 8][:, ib % 8, :]

            # xa^T = (x @ A^T)^T  [rank, tok]
            xaT = const.tile([RANK, TPC], F32R)
            for q in range(2):
                ps_xa = mp.tile([RANK, 512], FP32, tag="mp", name=f"psxa{q}")
                for ib in range(NI):
                    nc.tensor.matmul(
                        ps_xa[:], aT[:, ib, :], xT(ib)[:, q * 512:(q + 1) * 512],
                        start=(ib == 0), stop=(ib == NI - 1))
                nc.vector.tensor_copy(xaT[:, q * 512:(q + 1) * 512], ps_xa[:])

            # G = A @ A^T  [rank, rank]
            ps_g = mp.tile([RANK, RANK], FP32, tag="mp", name="psg")
            for ib in range(NI):
                nc.tensor.matmul(ps_g[:], aT[:, ib, :], aT[:, ib, :],
                                 start=(ib == 0), stop=(ib == NI - 1))
            g_sb = const.tile([RANK, RANK], F32R)
            nc.vector.tensor_copy(g_sb[:], ps_g[:])

            # n3d[o] = rowsum((B2 @ G) * B2)   [128, 32]
            n3d = const.tile([128, 32], FP32)
            scratch = const.tile([128, 144], FP32)
            for c in range(NCOL):
                b2tc = b2tp.tile([RANK, OC], F32R, tag="b2t", name=f"b2tn{c}")
                nc.sync.dma_start(b2tc[:], b2t_d[:, c * OC:(c + 1) * OC].bitcast(F32R))
                for s in range(4):
                    osub = c * 4 + s
                    ps_bg = mp.tile([128, RANK], FP32, tag="mp", name=f"psbg{osub}")
                    nc.tensor.matmul(ps_bg[:], b2tc[:, s * 128:(s + 1) * 128],
                                     g_sb[:], start=True, stop=True)
                    nc.vector.tensor_mul(scratch[:, :RANK], ps_bg[:],
                                         b2S[:, osub, :])
                    nc.vector.reduce_sum(n3d[:, osub:osub + 1],
                                         scratch[:, :RANK],
                                         axis=mybir.AxisListType.X)

            # per-subtile norm accumulators (SBUF); PSUM only holds one
            # h-chunk's partial at a time, keeping banks free across columns
            nacc = [const.tile([128, 144], FP32, name=f"nacc{s}")
                    for s in range(4)]
            nsq = const.tile([128, 4], FP32)
            nsq2 = const.tile([128, 4], FP32)
            nrm = const.tile([128, 4], FP32)
            rnrm = const.tile([128, 4], FP32)
            scol = const.tile([128, 4], FP32)
            srow = const.tile([4, 128], FP32)

            for c in range(ncol_limit):
                b2tc = b2tp.tile([RANK, OC], F32R, tag="b2t", name=f"b2tc{c}")
                nc.sync.dma_start(b2tc[:], b2t_d[:, c * OC:(c + 1) * OC].bitcast(F32R))

                outsb = []
                for h in range(NH):
                    wts = []
                    for j in range(H):
                        ib = h * H + j
                        w_t = wp.tile([128, OC], F32R, tag="w", name=f"w{c}_{ib}")
                        nc.sync.dma_start(w_t[:], wt_d[c, ib].bitcast(F32R))
                        wts.append(w_t)
                    # fused norm matmuls: gram diag (n1) + W@A^T (n2),
                    # one transient PSUM partial per (h, subtile)
                    for s in range(4):
                        ps_n = npp.tile([128, 144], FP32, tag="np",
                                        name=f"np{c}_{h}_{s}")
                        for j in range(H):
                            ib = h * H + j
                            w16s = w16p.tile([128, 144], FP16, tag="w16",
                                             name=f"w16_{c}_{ib}_{s}")
                            nc.scalar.activation(
                                w16s[:, 0:128],
                                wts[j][:, s * 128:(s + 1) * 128].bitcast(FP32),
                                mybir.ActivationFunctionType.Copy)
                            nc.gpsimd.tensor_copy(w16s[:, 128:144], a16[:, ib, :])
                            nc.tensor.matmul(ps_n[:], w16s[:, 0:128], w16s[:],
                                             start=(j == 0), stop=(j == H - 1))
                        if h == 0:
                            nc.vector.tensor_copy(nacc[s][:], ps_n[:])
                        else:
                            nc.vector.tensor_add(nacc[s][:], nacc[s][:], ps_n[:])
                    for t in range(NT):
                        ps_m = mp.tile([128, OC], FP32, tag="mp",
                                       name=f"pm{c}_{h}_{t}")
                        for j in range(H):
                            ib = h * H + j
                            nc.tensor.matmul(
                                ps_m[:], xT(ib)[:, t * 128:(t + 1) * 128], wts[j][:],
                                start=(j == 0),
                                stop=(j == H - 1 and h != NH - 1))
                        if h == NH - 1:
                            # rank-16 DoRA term folded into the accumulation
                            nc.tensor.matmul(ps_m[:],
                                             xaT[:, t * 128:(t + 1) * 128],
                                             b2tc[:], start=False, stop=True)
                        if h == 0:
                            o_t = outp.tile([128, OC], FP32, tag="o",
                                            name=f"o{c}_{t}")
                            outsb.append(o_t)
                            nc.vector.tensor_copy(o_t[:], ps_m[:])
                        else:
                            nc.vector.tensor_add(outsb[t][:], outsb[t][:], ps_m[:])

                # finish norms for this column -> scale row -> broadcast
                for s in range(4):
                    osub = c * 4 + s
                    nc.vector.tensor_mul(scratch[:, :128], nacc[s][:, 0:128],
                                         ident[:])
                    nc.vector.reduce_sum(nsq[:, s:s + 1], scratch[:, :128],
                                         axis=mybir.AxisListType.X)
                    nc.vector.tensor_mul(scratch[:, :RANK], nacc[s][:, 128:144],
                                         b2S[:, osub, :])
                    nc.vector.reduce_sum(nsq2[:, s:s + 1], scratch[:, :RANK],
                                         axis=mybir.AxisListType.X)
                nc.vector.tensor_scalar_mul(nsq2[:], nsq2[:], 2.0)
                nc.vector.tensor_add(nsq2[:], nsq2[:], nsq[:])
                nc.vector.tensor_add(nsq2[:], nsq2[:], n3d[:, c * 4:(c + 1) * 4])
                nc.scalar.activation(nrm[:], nsq2[:],
                                     mybir.ActivationFunctionType.Sqrt)
                nc.vector.reciprocal(rnrm[:], nrm[:])
                nc.vector.tensor_mul(scol[:], rnrm[:], magS[:, c * 4:(c + 1) * 4])
                ps_t = mp.tile([4, 128], FP32, tag="mp", name=f"pst{c}")
                nc.tensor.transpose(ps_t[:], scol[:], ident[:])
                nc.vector.tensor_copy(srow[:], ps_t[:])
                sbc = sbcp.tile([128, OC], FP32, tag="sbc", name=f"sbc{c}")
                nc.sync.dma_start(srow_d[c:c + 1, :], srow[:])
                _sl = srow_d[c:c + 1, :]
                srow_bcast = bass.AP(
                    tensor=_sl.tensor, offset=_sl.offset,
                    ap=[[0, 128], [1, OC]])
                nc.gpsimd.dma_start(sbc[:], srow_bcast)

                for t in range(NT):
                    nc.vector.tensor_mul(outsb[t][:], outsb[t][:], sbc[:])
                    nc.sync.dma_start(
                        out_d[t * 128:(t + 1) * 128, c * OC:(c + 1) * OC],
                        outsb[t][:])

    nc.compile()
    return nc


_PROGRAM = None


def _get_program():
    global _PROGRAM
    if _PROGRAM is None:
        _PROGRAM = _build_program()
    return _PROGRAM


def _prep_inputs(x, weight, lora_a_w, lora_b_w, magnitude):
    xr = _round_f32r(x.reshape(TOK, DIN))
    wr = _round_f32r(weight)
    ar = _round_f32r(lora_a_w)
    b2 = _round_f32r(SCALING * lora_b_w.astype(np.float32))

    wT = np.ascontiguousarray(wr.T)                        # [in, out]
    wt = np.ascontiguousarray(
        wT.reshape(NI, 128, NCOL, OC).transpose(2, 0, 1, 3))
    at = np.ascontiguousarray(ar.T.reshape(NI, 128, RANK).transpose(1, 0, 2))
    b2r = np.ascontiguousarray(b2.reshape(32, 128, RANK).transpose(1, 0, 2))
    b2t = np.ascontiguousarray(b2.T)
    magr = np.ascontiguousarray(
        magnitude.astype(np.float32).reshape(32, 128).T)

    xTfull = xr.T                                           # [in, tok]
    in_maps = []
    for cpu in range(NCORES):
        xs = xTfull[:, cpu * TPC:(cpu + 1) * TPC]
        xt = np.ascontiguousarray(
            xs.reshape(NI, 128, TPC).transpose(1, 0, 2))
        in_maps.append({"xt": xt, "wt": wt, "at": at, "b2": b2r,
                        "b2t": b2t, "mag": magr})
    return in_maps


def kernel(x, weight, lora_a_w, lora_b_w, magnitude, _trace=False, **_kw):
    nc = _get_program()
    in_maps = _prep_inputs(x, weight, lora_a_w, lora_b_w, magnitude)
    res = run_bass_kernel_spmd(nc, in_maps, list(range(NCORES)), trace=_trace)
    out = np.concatenate([res.results[c]["out"] for c in range(NCORES)], axis=0)
    if _trace:
        kernel._last_results = res
    return out.reshape(4, 2048, DOUT)
